# revision 34
# baseline (speedup 1.0000x reference)
"""Trainium2 Bass kernel for: freq-domain Butterworth mask -> 3x3 conv ->
BatchNorm(train) -> SiLU, concat(act, freq).

Sharding: data-parallel over batch (2 images per core, 8 cores). BN statistics
are all-reduced across cores with an in-kernel AllReduce collective.

Layout: SBUF partitions = (img_local, channel) -> 128 partitions.
Conv = 9 shifted matmuls (taps) accumulating in PSUM; the two local images run
as concurrent PE tile-position pairs (0,0)/(64,64) in f16 (full PE rate,
fp32 PSUM accumulate). Conv output y stays resident in SBUF as f16 between
the stats pass and the normalization pass; BN stats use bn_stats/bn_aggr on
the fp32 PSUM accumulators and one tiny AllReduce.

Over the original baseline:
- halo reuse: each x row is DMA'd from HBM exactly once; the 2-row overlap of
  each conv strip is copied (f16, vector) from the previous strip's tile.
- dm broadcast covers only the new rows of each strip (~8 instead of 10).
- unpadded 256-wide staging tiles: x-load / freq-out / dm DMAs move 4-8KB
  contiguous descriptors (pad columns live only in the f16 conv input tile,
  memset once per rotating buffer).
- f16 conversion for the PE runs on vector (4 elem/cycle) instead of gpsimd;
  conv weights arrive from the host already in f16.
- bn_stats reads the fp32 PSUM directly, decoupled from the scalar y-copy.
- AllReduce payload folded to [64,2]; post-AR finalize is an Activation-engine
  func(in*scale+bias) chain, so the only AR-gated vector instruction is the
  reciprocal. Bulk DMAs are always issued from the sync engine (Act-issued
  DMAs and DVE PSUM copies measured 3-4x slower end-to-end).
"""

import numpy as np

B, C, H, W = 16, 64, 256, 256
N_CORES = 8
B_LOC = B // N_CORES          # images per core
SR = 8                        # strip rows (output rows per strip)
NSTRIP = H // SR
CUTOFF_L = 90.0
BN_EPS = 1e-5
NTOT = B * H * W              # BN stat count per channel
WP = W + 2                    # padded width

_CACHE = {}
MODE = "full"   # BN stats: "full" = AllReduce across cores, "p2local" = per-core


def _emit_body(nc, tc, pools_tag, dram_io, mode='full'):
    import concourse.bass as bass  # noqa: F401
    from concourse import mybir

    F32 = mybir.dt.float32
    F16 = mybir.dt.float16
    AF = mybir.ActivationFunctionType
    x_d, wt_d, dmh_d, gam_d, bet_d, out_d = dram_io

    from contextlib import ExitStack
    ctx = ExitStack()
    with ctx:
        persist = ctx.enter_context(tc.tile_pool(name=f"persist{pools_tag}", bufs=1))
        xp_p = ctx.enter_context(tc.tile_pool(name=f"xp{pools_tag}", bufs=3))
        freq_p = ctx.enter_context(tc.tile_pool(name=f"freq{pools_tag}", bufs=2))
        dm_p = ctx.enter_context(tc.tile_pool(name=f"dm{pools_tag}", bufs=2))
        psum_p = ctx.enter_context(tc.tile_pool(name=f"ps{pools_tag}", bufs=2, space="PSUM"))
        out_p = ctx.enter_context(tc.tile_pool(name=f"out{pools_tag}", bufs=3))
        dram_p = ctx.enter_context(tc.tile_pool(name=f"dram{pools_tag}", bufs=1, space="DRAM"))

        # ---- persistent tiles ----
        y_sb = persist.tile([128, H * W // SR // 4 * NSTRIP], F16)  # [128, 65536] f16
        assert y_sb.shape[1] == B_LOC * C * H * W // 128
        stats6 = persist.tile([128, NSTRIP * 4, 6], F32)
        mv_t = persist.tile([128, 2], F32)
        msq128 = persist.tile([128, 1], F32)
        w_sb = persist.tile([128, 9, C], F16)
        gam_t = persist.tile([64, 1], F32)
        bet_t = persist.tile([64, 1], F32)
        eps_t = persist.tile([64, 1], F32)
        stats_sb = persist.tile([128, 2], F32)
        t0_t = persist.tile([64, 2], F32)
        t1_t = persist.tile([64, 2], F32)
        msq_t = persist.tile([64, 1], F32)
        var_t = persist.tile([64, 1], F32)
        tmp_t = persist.tile([64, 1], F32)
        s2_full = persist.tile([128, 1], F32)
        b2_full = persist.tile([128, 1], F32)

        # ---- load weights (both partition halves, f16), per-channel params ----
        for img in range(2):
            nc.sync.dma_start(out=w_sb[img * 64:(img + 1) * 64, :, :], in_=wt_d[:, :, :])
        nc.sync.dma_start(out=gam_t[:, :], in_=gam_d[:, :])
        nc.sync.dma_start(out=bet_t[:, :], in_=bet_d[:, :])
        nc.vector.memset(eps_t[:, :], BN_EPS)

        # ---- pass 1: freq = x*dm, conv via 9 taps, y -> SBUF f16, stats ----
        # freq16 tile row t <-> image row r0-1+t (t=0..SR+1)
        prev_freq = None
        for s in range(NSTRIP):
            r0 = s * SR
            # new image rows this strip (each row loaded exactly once)
            a0 = 0 if s == 0 else r0 + 1
            a1 = min(r0 + SR + 1, H)
            nrows = a1 - a0          # 9 (s=0), 8 (middle), 7 (last)
            t0 = a0 - (r0 - 1)       # first freq16 row filled from x_t

            freq16 = freq_p.tile([128, SR + 2, WP], F16, tag="freq")
            x_t = xp_p.tile([128, SR + 1, W], F32, tag="xs")
            dm_t = dm_p.tile([128, SR + 1, W], F16, tag="dm")

            # load the new x rows (unpadded tiles -> 8KB contiguous descriptors)
            for img in range(B_LOC):
                nc.sync.dma_start(
                    out=x_t[img * 64:(img + 1) * 64, 0:nrows, :],
                    in_=x_d[img, :, a0:a1, :],
                )

            # dm rows [a0, a1) broadcast to all partitions (new rows only)
            nc.sync.dma_start(
                out=dm_t[:, 0:nrows, :],
                in_=dmh_d[a0:a1, :].unsqueeze(0).to_broadcast((128, nrows, W)),
            )

            # freq = x * dm in place (f32, feeds the freq output half)
            nc.vector.tensor_tensor(
                out=x_t[:, 0:nrows, :], in0=x_t[:, 0:nrows, :],
                in1=dm_t[:, 0:nrows, :], op=mybir.AluOpType.mult,
            )
            # f16 image for the TensorEngine; pad cols memset on first use of
            # each rotating buffer, halo rows via copy/memset
            nc.vector.tensor_copy(freq16[:, t0:t0 + nrows, 1:W + 1], x_t[:, 0:nrows, :])
            if s < 2:
                nc.vector.memset(freq16[:, :, 0:1], 0.0)
                nc.vector.memset(freq16[:, :, W + 1:W + 2], 0.0)
            if s == 0:
                nc.vector.memset(freq16[:, 0:1, 1:W + 1], 0.0)
            else:
                nc.vector.tensor_copy(freq16[:, 0:2, 1:W + 1], prev_freq[:, SR:SR + 2, 1:W + 1])
            if s == NSTRIP - 1:
                nc.vector.memset(freq16[:, SR + 1:SR + 2, 1:W + 1], 0.0)
            prev_freq = freq16

            # freq -> output channels [C:2C] (rows a0..a1, written once each)
            for img in range(B_LOC):
                nc.sync.dma_start(
                    out=out_d[img, C:2 * C, a0:a1, :],
                    in_=x_t[img * 64:(img + 1) * 64, 0:nrows, :],
                )

            if mode == "dma":
                continue
            # conv: SR/2 rounds of N=512 (2 output rows), 9 taps x 2 images
            ps = psum_p.tile([128, SR // 2 * 512], mybir.dt.float32, tag="ps")
            for rnd in range(SR // 2):
                for tap in range(9):
                    dy, dx = tap // 3, tap % 3
                    for img in range(B_LOC):
                        p0 = img * 64
                        out_ap = ps[p0:p0 + 64, rnd * 512:(rnd + 1) * 512] \
                            .rearrange("p (a b) -> p a b", a=2)
                        nc.tensor.matmul(
                            out_ap,
                            w_sb[p0:p0 + 64, tap, :],
                            freq16[p0:p0 + 64, 2 * rnd + dy:2 * rnd + dy + 2, dx:dx + W],
                            start=(tap == 0), stop=(tap == 8),
                            tile_position=(p0, p0),
                        )

            # y chunk -> SBUF f16; per-chunk BN stats from exact fp32 PSUM
            y_chunk = y_sb[:, s * (SR // 2 * 512):(s + 1) * (SR // 2 * 512)]
            nc.scalar.activation(out=y_chunk, in_=ps[:, :], func=AF.Copy)
            for j in range(SR // 2):
                nc.vector.bn_stats(
                    out=stats6[:, s * (SR // 2) + j, :],
                    in_=ps[:, j * 512:(j + 1) * 512],
                )

        if mode == "dma":
            # fake act-out DMA so the DMA pattern matches pass 2 volume
            for k2 in range(H // SR):
                for img in range(B_LOC):
                    nc.sync.dma_start(
                        out=out_d[img, 0:C, k2 * SR:k2 * SR + SR, :],
                        in_=x_t[img * 64:(img + 1) * 64, 0:SR, :],
                    )
            return
        # ---- BN stats: aggregate, fold to [64,2], AllReduce, finalize ----
        # Per-core partial sums stay unscaled: all 16 (img, core) shards have
        # equal count, so the post-AR scale is just 1/16.
        NSH = 2 * N_CORES
        nc.vector.bn_aggr(out=mv_t[:, :], in_=stats6[:, :, :])  # (mean, var) per part
        nc.vector.tensor_mul(msq128[:, :], mv_t[:, 0:1], mv_t[:, 0:1])
        nc.vector.tensor_add(stats_sb[:, 1:2], mv_t[:, 1:2], msq128[:, :])
        nc.vector.tensor_copy(stats_sb[:, 0:1], mv_t[:, 0:1])
        # fold img2 partitions onto img1: t1 <- parts 64:128, add on vector
        nc.sync.dma_start(out=t1_t[:, :], in_=stats_sb[64:128, :])
        nc.vector.tensor_add(t0_t[:, :], stats_sb[0:64, :], t1_t[:, :])
        if mode == "p2local":
            # timing ablation: skip the collective, use local stats
            t_ap = t0_t
            scl = float(N_CORES) / NSH
        else:
            ar_in = dram_p.tile([64, 2], F32)
            ar_out = dram_p.tile([64, 2], F32)
            nc.sync.dma_start(out=ar_in[:, :], in_=t0_t[:, :])
            nc.gpsimd.collective_compute(
                "AllReduce", mybir.AluOpType.add,
                replica_groups=[list(range(N_CORES))],
                ins=[ar_in.opt()], outs=[ar_out.opt()],
            )
            # gated loads go through gpsimd so the SP stream never stalls
            nc.gpsimd.dma_start(out=t1_t[:, :], in_=ar_out[:, :])
            t_ap = t1_t
            scl = 1.0 / NSH
        # Post-AR finalize: entirely on the Activation engine (the gated
        # stream), via func(in*scale + bias) chains.
        nc.scalar.activation(out=tmp_t[:, :], in_=t_ap[:, 0:1], func=AF.Copy,
                             scale=scl)                      # mean
        nc.scalar.activation(out=msq_t[:, :], in_=t_ap[:, 0:1], func=AF.Square,
                             scale=scl)                      # mean^2
        nc.scalar.activation(out=var_t[:, :], in_=msq_t[:, :], func=AF.Identity,
                             scale=-1.0, bias=eps_t[:, :])   # eps - mean^2
        nc.scalar.activation(out=var_t[:, :], in_=t_ap[:, 1:2], func=AF.Sqrt,
                             scale=scl, bias=var_t[:, :])    # sqrt(var + eps)
        nc.vector.reciprocal(out=var_t[:, :], in_=var_t[:, :])  # rstd
        nc.scalar.activation(out=s2_full[0:64, :], in_=var_t[:, :], func=AF.Copy,
                             scale=gam_t[:, 0:1])            # s2 = rstd*gamma
        # stats are over pre-bias y, so the conv bias cancels:
        # b2 = beta - mean_pre * s2
        nc.scalar.activation(out=tmp_t[:, :], in_=tmp_t[:, :], func=AF.Copy,
                             scale=s2_full[0:64, 0:1])       # mean*s2
        nc.scalar.activation(out=b2_full[0:64, :], in_=tmp_t[:, :], func=AF.Identity,
                             scale=-1.0, bias=bet_t[:, :])
        nc.sync.dma_start(out=s2_full[64:128, :], in_=s2_full[0:64, :])
        nc.sync.dma_start(out=b2_full[64:128, :], in_=b2_full[0:64, :])

        if mode not in ("full", "p2local"):
            if mode == "nopass2":
                # keep the stats/collective chain and y_sb writes live
                nc.vector.reduce_sum(out=msq128[:, 0:1], in_=y_sb[:, 0:512],
                                     axis=mybir.AxisListType.X)
                nc.vector.tensor_add(stats_sb[:, 0:1], s2_full[:, :], b2_full[:, :])
                nc.vector.tensor_add(stats_sb[:, 1:2], msq128[:, :], b2_full[:, :])
                nc.sync.dma_start(out=out_d[0, 0, 0, 0:2], in_=stats_sb[0:1, :].squeeze(0))
            return
        # ---- pass 2: act = SiLU(y * s2 + b2) -> output channels [0:C] ----
        CHUNK = 1024                       # 4 output rows per chunk
        n_chunks = (B_LOC * C * H * W // 128) // CHUNK
        for k in range(n_chunks):
            r0 = k * (CHUNK // W)  # 4 output rows per chunk
            o_t = out_p.tile([128, CHUNK], F32, tag="o")
            nc.scalar.activation(
                out=o_t[:, :], in_=y_sb[:, k * CHUNK:(k + 1) * CHUNK],
                func=AF.Silu, scale=s2_full[:, 0:1], bias=b2_full[:, 0:1],
            )
            for img in range(B_LOC):
                nc.sync.dma_start(
                    out=out_d[img, 0:C, r0:r0 + 4, :],
                    in_=o_t[img * 64:(img + 1) * 64, :],
                )


def _build(repeat=1, mode=None):
    if mode is None:
        mode = MODE
    key = ("nc", repeat, mode)
    if key in _CACHE:
        return _CACHE[key]
    import concourse.bacc as bacc
    import concourse.tile as tile
    from concourse import mybir

    F32 = mybir.dt.float32
    F16 = mybir.dt.float16

    nc = bacc.Bacc("TRN2", target_bir_lowering=False, debug=False, num_devices=N_CORES)
    x_d = nc.dram_tensor("x", [B_LOC, C, H, W], F32, kind="ExternalInput")
    wt_d = nc.dram_tensor("wt", [C, 9, C], F16, kind="ExternalInput")
    dmh_d = nc.dram_tensor("dmh", [H, W], F16, kind="ExternalInput")
    gam_d = nc.dram_tensor("gamma_in", [C, 1], F32, kind="ExternalInput")
    bet_d = nc.dram_tensor("beta_in", [C, 1], F32, kind="ExternalInput")
    out_d = nc.dram_tensor("out", [B_LOC, 2 * C, H, W], F32, kind="ExternalOutput")
    dram_io = (x_d, wt_d, dmh_d, gam_d, bet_d, out_d)

    with tile.TileContext(nc) as tc:
        for rep in range(repeat):
            _emit_body(nc, tc, rep, dram_io, mode=mode)
    nc.compile()
    _CACHE[key] = nc
    return nc


def _host_inputs(x, conv_w, conv_b, gamma, beta):
    # dm exactly as the reference builds it (fp32), then f16 + zero pad cols
    u = (np.arange(H, dtype=np.float32) - H // 2)[:, None]
    v = (np.arange(W, dtype=np.float32) - W // 2)[None, :]
    d = np.sqrt(u * u + v * v)
    d = np.where(d == 0, np.float32(1e-6), d)
    filt = 1.0 / (1.0 + (d / np.float32(CUTOFF_L)) ** 2)
    dm = (0.8 * filt + 0.5).astype(np.float32)
    dmh = dm.astype(np.float16)

    wt = np.ascontiguousarray(np.transpose(conv_w, (1, 2, 3, 0)).reshape(C, 9, C))

    shared = {
        "wt": wt.astype(np.float16),
        "dmh": dmh,
        "gamma_in": gamma.reshape(C, 1).astype(np.float32),
        "beta_in": beta.reshape(C, 1).astype(np.float32),
    }
    in_maps = []
    for c in range(N_CORES):
        m = dict(shared)
        m["x"] = np.ascontiguousarray(x[c * B_LOC:(c + 1) * B_LOC]).astype(np.float32)
        in_maps.append(m)
    return in_maps


def kernel(x, conv_w, conv_b, gamma, beta):
    from concourse.bass_utils import run_bass_kernel_spmd

    x = np.asarray(x)
    nc = _build(repeat=1)
    in_maps = _host_inputs(x, np.asarray(conv_w), np.asarray(conv_b),
                           np.asarray(gamma), np.asarray(beta))
    res = run_bass_kernel_spmd(nc, in_maps, core_ids=list(range(N_CORES)))
    out = np.concatenate([res.results[c]["out"] for c in range(N_CORES)], axis=0)
    return out.astype(np.float32)
